# revision 14
# baseline (speedup 1.0000x reference)
"""Trainium2 Bass kernel for nn_Cross_Attention (B=16, C=256, H=W=96).

reference:
    q = Z1.reshape(B, C, N); k = Zr.reshape(B, C, N)         # N = H*W
    energy    = q @ k^T                                       # [B, C, C]
    attention = softmax(rowmax(energy) - energy, axis=-1)
    out       = attention @ k                                 # [B, C, N]
    return beta * out + Zr
ref absmax ~5.4, tol 2e-2 -> bf16 I/O rounding (~4e-3) is well inside it.

Strategy: data-parallel over batch, 2 batches per NeuronCore on 8 cores.
All HBM I/O in bf16: q^T host-packed [P, NT, C] partition-major so the
N-contraction matmul streams straight from DRAM, Zr host-downcast to bf16
(it is k, the residual, and the mm2 rhs all at once), and the output is
stored bf16 then upcast on host.  k^T for the energy matmul is produced
on-chip tile-by-tile on the TensorE (transpose-mode matmul) so k crosses
HBM exactly once.  softmax(max - e) == exp(min - e) / sum(exp(min - e))
row-wise: only a row-min is needed, exp args are always <= 0 (no
overflow), the sum is >= 1 (no div-by-0).  The residual Zr IS k, so
beta*out + Zr == (beta*attention + I) @ k: beta and 1/sum are folded into
the attention weights and I is added to their diagonal block, making the
second matmul produce the final output directly in PSUM (bitwise bf16(Zr)
when beta == 0).  PSUM->SBUF downcast copies alternate between the DVE
and Activation engines so neither gates the TensorE; batch b's stores are
queued on the sync ring BEHIND batch b+1's loads so the store burst never
steals DMA bandwidth from the load stream that feeds the PE.
"""

from contextlib import ExitStack

import ml_dtypes
import numpy as np

import concourse.bass as bass
import concourse.tile as tile
from concourse import bacc, mybir
from concourse.bass_utils import run_bass_kernel_spmd
from concourse.masks import make_identity

B, C, H, W = 16, 256, 96, 96
N = H * W                    # 9216
P = 128
NCORES = 8
BL = B // NCORES             # 2 batches per core
CT = C // P                  # 2 c-tiles of 128
NT = N // P                  # 72 contraction tiles for energy
TCH = 18                     # qt tiles per DMA chunk
NCH = NT // TCH              # 4 chunks (last one split per c-tile)
TQT = (NCH - 1) * TCH        # 54 t-tiles in the interleaved qt tensor
NH = N // 2                  # 4608: kb slice width (half a c-tile row)
NQ = N // 4                  # 2304: zr tile width (quarter c-tile row)
TPH = NH // P                # 36 n-tiles per h-half
OW = 512                     # mm2 psum chunk width == one full PSUM bank
WPH = NH // OW               # 9 psum chunks per h-half
SW = 3 * OW                  # 1536: store piece width (3 per h-half)

F32 = mybir.dt.float32
BF16 = mybir.dt.bfloat16


def _build_program():
    nc = bacc.Bacc("TRN2", target_bir_lowering=False, debug=False,
                   num_devices=NCORES)

    qt_ext = nc.dram_tensor("qt", [BL, P, TQT, C], BF16, kind="ExternalInput")
    qtt_ext = nc.dram_tensor("qtt", [BL, CT, P, TCH, P], BF16,
                             kind="ExternalInput")
    zr_ext = nc.dram_tensor("zr", [BL, C, N], BF16, kind="ExternalInput")
    beta_ext = nc.dram_tensor("beta", [1], F32, kind="ExternalInput")
    out_ext = nc.dram_tensor("out", [BL, C, N], BF16, kind="ExternalOutput")

    with tile.TileContext(nc) as tc, ExitStack() as ctx:
        qtp = ctx.enter_context(tc.tile_pool(name="qtp", bufs=4))
        kbp = ctx.enter_context(tc.tile_pool(name="kbp", bufs=8))
        kttp = ctx.enter_context(tc.tile_pool(name="kttp", bufs=4))
        expp = ctx.enter_context(tc.tile_pool(name="expp", bufs=2))
        attp = ctx.enter_context(tc.tile_pool(name="attp", bufs=2))
        atTp = ctx.enter_context(tc.tile_pool(name="atTp", bufs=2))
        outbp = ctx.enter_context(tc.tile_pool(name="outbp", bufs=6))
        statp = ctx.enter_context(tc.tile_pool(name="statp", bufs=8))
        singles = ctx.enter_context(tc.tile_pool(name="singles", bufs=1))
        engp = ctx.enter_context(tc.tile_pool(name="engp", bufs=2, space="PSUM"))
        trp = ctx.enter_context(tc.tile_pool(name="trp", bufs=3, space="PSUM"))
        outp = ctx.enter_context(tc.tile_pool(name="outp", bufs=3, space="PSUM"))

        ident = singles.tile([P, P], BF16)
        make_identity(nc, ident)
        beta_sb = singles.tile([P, 1], F32)
        nc.gpsimd.dma_start(out=beta_sb, in_=beta_ext.ap().to_broadcast((P, 1)))

        # tl index groups per chunk: [0-3],[4-7],[8-11],[12-15],[16-17] --
        # each fits one 2KB psum bank; copies alternate Activation/DVE so
        # neither engine's serial copy stream gates the energy matmuls
        GROUPS = [list(range(g * 4, min(g * 4 + 4, TCH)))
                  for g in range((TCH + 3) // 4)]

        def emit_tr_group(i, h, g, kb, ktts):
            tls = GROUPS[g]
            tr = trp.tile([P, 4, CT, P], BF16, name="tr4")
            for j, tl in enumerate(tls):
                th = i * TCH + tl - h * TPH
                for dj in range(CT):
                    nc.tensor.transpose(tr[:, j, dj, :],
                                        kb[dj, h][:, th * P:(th + 1) * P],
                                        ident)
            ktt = kttp.tile([P, 4, CT * P], BF16, name="ktt4")
            n = len(tls)
            if (g + i) % 2 == 0:
                nc.scalar.copy(out=ktt[:, :n, :], in_=tr[:, :n, :, :])
            else:
                nc.vector.tensor_copy(out=ktt[:, :n, :], in_=tr[:, :n, :, :])
            for j, tl in enumerate(tls):
                ktts[tl] = ktt[:, j, :]

        deferred_stores = []
        for b in range(BL):
            # ---- interleaved load/compute pipeline: chunk i of the
            # energy matmul consumes zr quarter i (straight into the kb
            # halves, bf16 on the wire) and qt chunk i (in two half loads
            # so matmuls start before the whole chunk lands) ----
            kb = {}
            eng = [engp.tile([P, C], F32, name="eng") for _ in range(CT)]
            for i in range(NCH - 1):
                h, qq = divmod(i, 2)
                qt_t = qtp.tile([P, TCH, C], BF16)
                hf = TCH // 2
                if i == 0 and b == 0:
                    # kernel head: half-granularity loads, qt half slotted
                    # between them, so the first transposes start ~2us
                    # earlier and the PE clock starts ramping sooner
                    nh2 = NQ // 2
                    for cj in range(CT):
                        kb[cj, h] = kbp.tile([P, NH], BF16, name="kb_t")
                        nc.sync.dma_start(
                            out=kb[cj, h][:, :nh2],
                            in_=zr_ext[b, cj * P:(cj + 1) * P, :nh2])
                    nc.sync.dma_start(out=qt_t[:, :hf, :],
                                      in_=qt_ext[b, :, :hf, :])
                    for cj in range(CT):
                        nc.sync.dma_start(
                            out=kb[cj, h][:, nh2:NQ],
                            in_=zr_ext[b, cj * P:(cj + 1) * P, nh2:NQ])
                    nc.sync.dma_start(out=qt_t[:, hf:, :],
                                      in_=qt_ext[b, :, hf:TCH, :])
                else:
                    for cj in range(CT):
                        if qq == 0:
                            kb[cj, h] = kbp.tile([P, NH], BF16, name="kb_t")
                        nc.sync.dma_start(
                            out=kb[cj, h][:, qq * NQ:(qq + 1) * NQ],
                            in_=zr_ext[b, cj * P:(cj + 1) * P,
                                       i * NQ:(i + 1) * NQ],
                        )
                    nc.sync.dma_start(out=qt_t[:, :hf, :],
                                      in_=qt_ext[b, :, i * TCH:i * TCH + hf, :])
                    nc.sync.dma_start(out=qt_t[:, hf:, :],
                                      in_=qt_ext[b, :, i * TCH + hf:(i + 1) * TCH, :])
                # pipelined transpose-group / matmul-group emission with a
                # lookahead of 2, so only 3 psum transpose bufs are live
                # and the PE always has matmul work while copies land
                ktts = [None] * TCH
                emit_tr_group(i, h, 0, kb, ktts)
                emit_tr_group(i, h, 1, kb, ktts)
                for g in range(len(GROUPS)):
                    if g + 2 < len(GROUPS):
                        emit_tr_group(i, h, g + 2, kb, ktts)
                    for tl in GROUPS[g]:
                        t = i * TCH + tl
                        for ci in range(CT):
                            nc.tensor.matmul(
                                eng[ci],
                                lhsT=qt_t[:, tl, ci * P:(ci + 1) * P],
                                rhs=ktts[tl],
                                start=(t == 0),
                                stop=False,
                            )

            # ---- final chunk, split per c-tile: eng[0] closes a full qt
            # sub-load earlier than eng[1], so its softmax / mm2 / stores
            # overlap the ci=1 stream ----
            i = NCH - 1
            h, qq = divmod(i, 2)
            for cj in range(CT):
                nc.sync.dma_start(
                    out=kb[cj, h][:, qq * NQ:(qq + 1) * NQ],
                    in_=zr_ext[b, cj * P:(cj + 1) * P, i * NQ:(i + 1) * NQ],
                )
            ktts = [None] * TCH
            qtt_ts = []
            for ci in range(CT):
                qtt_t = qtp.tile([P, TCH, P], BF16, name="qtt_t", tag="qt_t")
                nc.sync.dma_start(out=qtt_t, in_=qtt_ext[b, ci])
                qtt_ts.append(qtt_t)
            emit_tr_group(i, h, 0, kb, ktts)
            emit_tr_group(i, h, 1, kb, ktts)
            for g in range(len(GROUPS)):
                if g + 2 < len(GROUPS):
                    emit_tr_group(i, h, g + 2, kb, ktts)
                for tl in GROUPS[g]:
                    t = i * TCH + tl
                    nc.tensor.matmul(
                        eng[0],
                        lhsT=qtt_ts[0][:, tl, :],
                        rhs=ktts[tl],
                        start=False,
                        stop=(t == NT - 1),
                    )
            for tl in range(TCH):
                t = i * TCH + tl
                nc.tensor.matmul(
                    eng[1],
                    lhsT=qtt_ts[1][:, tl, :],
                    rhs=ktts[tl],
                    start=False,
                    stop=(t == NT - 1),
                )

            # previous batch's stores, queued on the sync ring BEHIND this
            # batch's loads: they drain in the DMA gap while this batch's
            # mm2 runs, never contending with the loads that feed the PE
            for dst_ap, src_t in deferred_stores:
                nc.sync.dma_start(out=dst_ap, in_=src_t)
            deferred_stores = []

            # ---- softmax(max-e) = exp(min-e)/sum; fold beta/sum in.
            # Per-ci attnT tiles keep mm2(ci=0) independent of softmax(1) ----
            attnT = []
            for ci in range(CT):
                mn = statp.tile([P, 1], F32)
                nc.vector.tensor_reduce(out=mn, in_=eng[ci],
                                        axis=mybir.AxisListType.X,
                                        op=mybir.AluOpType.min)
                ex = expp.tile([P, C], F32)
                sm = statp.tile([P, 1], F32)
                nc.scalar.activation(out=ex, in_=eng[ci],
                                     func=mybir.ActivationFunctionType.Exp,
                                     bias=mn, scale=-1.0, accum_out=sm)
                rc = statp.tile([P, 1], F32)
                nc.vector.reciprocal(out=rc, in_=sm)
                rb = statp.tile([P, 1], F32)
                nc.vector.tensor_mul(out=rb, in0=rc, in1=beta_sb)
                at = attp.tile([P, C], BF16)
                nc.vector.tensor_scalar_mul(out=at, in0=ex, scalar1=rb)
                # residual fold: out = (beta*A + I) @ k, so add I to the
                # diagonal block of this ci's attention rows
                nc.vector.tensor_add(out=at[:, ci * P:(ci + 1) * P],
                                     in0=at[:, ci * P:(ci + 1) * P],
                                     in1=ident)
                trA = trp.tile([P, CT, P], BF16, name="trA", tag="tr4")
                for dj in range(CT):
                    nc.tensor.transpose(trA[:, dj, :],
                                        at[:, dj * P:(dj + 1) * P], ident)
                atT = atTp.tile([P, CT, P], BF16, name="atT")
                nc.vector.tensor_copy(out=atT, in_=trA)
                attnT.append(atT)

            # ---- out = (beta*A + I) @ k: psum holds the final values;
            # downcast copies alternate DVE/Activation, stores stream out
            # in 1536-wide pieces as soon as each 3 chunks are copied ----
            for ci in range(CT):
                for h in range(2):
                    ot = outbp.tile([P, NH], BF16, name="ot")
                    for w in range(WPH):
                        ps = outp.tile([P, OW], F32)
                        for dj in range(CT):
                            nc.tensor.matmul(
                                ps,
                                lhsT=attnT[ci][:, dj, :],
                                rhs=kb[dj, h][:, w * OW:(w + 1) * OW],
                                start=(dj == 0),
                                stop=(dj == CT - 1),
                            )
                        if w % 2 == 0:
                            nc.vector.tensor_copy(
                                out=ot[:, w * OW:(w + 1) * OW], in_=ps)
                        else:
                            nc.scalar.copy(
                                out=ot[:, w * OW:(w + 1) * OW], in_=ps)
                        if w % 3 == 2:
                            seg = w // 3
                            dst = out_ext[b, ci * P:(ci + 1) * P,
                                          h * NH + seg * SW:
                                          h * NH + (seg + 1) * SW]
                            src = ot[:, seg * SW:(seg + 1) * SW]
                            if b < BL - 1:
                                deferred_stores.append((dst, src))
                            elif (ci * 2 + h + seg) % 2 == 0:
                                # alternate rings so per-DMA init latencies
                                # overlap instead of serializing the tail
                                nc.sync.dma_start(out=dst, in_=src)
                            else:
                                nc.gpsimd.dma_start(out=dst, in_=src)

    nc.compile()
    return nc


_NC_CACHE = None


def _get_program():
    global _NC_CACHE
    if _NC_CACHE is None:
        _NC_CACHE = _build_program()
    return _NC_CACHE


def pack_qt(Z1):
    # bf16 q^T, partition-major: full[b, p, t, c] = q[b, c, t*128+p];
    # t < TQT interleaved-ci ("qt"), the last chunk split per ci ("qtt")
    x = Z1.reshape(B, C, NT, P).astype(ml_dtypes.bfloat16)
    full = x.transpose(0, 3, 2, 1)
    qta = np.ascontiguousarray(full[:, :, :TQT, :])
    qtb = np.ascontiguousarray(
        full[:, :, TQT:, :].reshape(B, P, TCH, CT, P).transpose(0, 3, 1, 2, 4))
    return qta, qtb


def kernel(Z1, Zr, beta):
    Z1 = np.asarray(Z1, dtype=np.float32)
    Zr = np.asarray(Zr, dtype=np.float32)
    beta = np.asarray(beta, dtype=np.float32).reshape(1)

    qta, qtb = pack_qt(Z1)
    zr = np.ascontiguousarray(
        Zr.reshape(B, C, N).astype(ml_dtypes.bfloat16))

    in_maps = []
    for i in range(NCORES):
        s = slice(i * BL, (i + 1) * BL)
        in_maps.append({"qt": qta[s], "qtt": qtb[s], "zr": zr[s],
                        "beta": beta})

    nc = _get_program()
    res = run_bass_kernel_spmd(nc, in_maps, list(range(NCORES)))
    out = np.concatenate([r["out"] for r in res.results], axis=0)
    return out.astype(np.float32).reshape(B, C, H, W)
